# revision 47
# baseline (speedup 1.0000x reference)
"""Trainium2 Bass kernel: causal multi-head attention with an extra time-mixing
matrix D (attn = D @ softmax(mask(Q K^T / sqrt(e))) @ V, concat heads, out proj).

Shapes (hardcoded): B=4, T=2048, d=1024, H=16, e=64, fp32 in/out.
Sharding over 8 NeuronCores: data-parallel over batch (4) x tensor-parallel over
heads (2 groups of 8). Each core computes its batch/head-group partial
y_part = concat(attn_heads) @ Wo_part^T; host sums the 2 partials per batch and
adds bo.

All operands are cast to bf16 on the host (tolerance 2e-2 >> bf16 error):
halves HBM traffic and lets small-moving-dim matmuls run at full rate.
PV runs in the [q, e] output orientation (stationary = exp-score slice,
moving = V) so the result lands pre-transposed for the D @ PV step.

The kernel is fully software-pipelined around the activation engine's exp
stream (the per-chunk serial resource): attention for pair 0 starts as soon
as V/QK projections for its first query block exist, and all remaining
work — V projection, per-pair QK projections (re-reading x from HBM, which
has slack), and D@PV of completed pairs — is drip-fed from a FIFO into the
score/exp chunk stream so the tensor engine never idles while exp runs.
Scores/exp/PV are trimmed to the causal region at 128-column granularity.
"""

import sys

for _p in ("/opt/trn_rl_repo", "/root/.axon_site/_ro/trn_rl_repo"):
    if _p not in sys.path:
        sys.path.append(_p)

from contextlib import ExitStack

import numpy as np

import concourse.bass as bass  # noqa: F401  (AP helpers)
import concourse.tile as tile
from concourse import bacc, mybir
from concourse.bass_utils import run_bass_kernel_spmd

dt = mybir.dt

B, T, D, H, E = 4, 2048, 1024, 16, 64
HG = 8          # heads per core (tensor-parallel group)
COEF = 1.0 / E ** 0.5
P = 128         # partitions
TQB = 512       # query-block width
NTQ = T // TQB  # 4 query blocks
NTC = T // P    # 16 time chunks
ND = D // P     # 8 contraction chunks (d)
EV = E + 1      # V columns per head incl. rowsum-ones column

_CACHED_NC = None


def _build_nc():
    """Build + compile the single-core program (same NEFF on all 8 cores)."""
    nc = bacc.Bacc("TRN2", target_bir_lowering=False, debug=False)
    f32, bf16 = dt.float32, dt.bfloat16
    Exp = mybir.ActivationFunctionType.Exp
    Ident = mybir.ActivationFunctionType.Identity
    mult = mybir.AluOpType.mult
    add_op = mybir.AluOpType.add

    xqT = nc.dram_tensor("xqT", [D + 1, T], bf16, kind="ExternalInput").ap()
    xkT = nc.dram_tensor("xkT", [D + 1, T], bf16, kind="ExternalInput").ap()
    xvT = nc.dram_tensor("xvT", [D + 1, T], bf16, kind="ExternalInput").ap()
    wqT = nc.dram_tensor("wqT", [D, 512], bf16, kind="ExternalInput").ap()
    wkT = nc.dram_tensor("wkT", [D, 512], bf16, kind="ExternalInput").ap()
    wvT = nc.dram_tensor("wvT", [D + 1, 512], bf16, kind="ExternalInput").ap()
    woT = nc.dram_tensor("woT", [512, D], bf16, kind="ExternalInput").ap()
    dTd = nc.dram_tensor("dTd", [T, T], bf16, kind="ExternalInput").ap()
    msk = nc.dram_tensor("msk", [P, 256], bf16, kind="ExternalInput").ap()
    bqk = nc.dram_tensor("bqk", [P, 8], f32, kind="ExternalInput").ap()
    bvt = nc.dram_tensor("bvt", [P, 512], bf16, kind="ExternalInput").ap()
    y = nc.dram_tensor("y", [T, D], f32, kind="ExternalOutput").ap()
    import os
    dbg = os.environ.get("KDEBUG") == "1"
    if dbg:
        dq = nc.dram_tensor("dq", [512, T], f32, kind="ExternalOutput").ap()
        dk = nc.dram_tensor("dk", [512, T], f32, kind="ExternalOutput").ap()
        dv = nc.dram_tensor("dv", [NTC * P, HG * EV], f32,
                            kind="ExternalOutput").ap()
        dpvg = nc.dram_tensor("dpvg", [NTC * P, 512], f32,
                              kind="ExternalOutput").ap()
        da2 = nc.dram_tensor("da2", [4 * NTQ * P, 512], f32,
                             kind="ExternalOutput").ap()

    def dram_chunks(ap_, nd, cols):
        """[nd*128, w] region of a DRAM tensor as a [128, nd, w] AP."""
        return ap_[0:P * nd, cols].rearrange("(d p) q -> p d q", p=P)

    with tile.TileContext(nc) as tc, ExitStack() as ctx:
        # ---- persistent constants -----------------------------------------
        consts = ctx.enter_context(tc.tile_pool(name="consts", bufs=1))
        zrow = consts.tile([1, 512], bf16, tag="zrow")
        nc.vector.memset(zrow[:], 0.0)
        woh = []     # wo tile, allocated once the projection pools close

        # ---- persistent activations ---------------------------------------
        proj = ctx.enter_context(tc.tile_pool(name="proj", bufs=1))
        qt = [proj.tile([P, T], bf16, tag=f"qt{p}", name=f"qt{p}") for p in range(4)]
        kt = [proj.tile([P, T], bf16, tag=f"kt{p}", name=f"kt{p}") for p in range(4)]
        vt = [proj.tile([P, HG * EV], bf16, tag=f"vt{t}", name=f"vt{t}")
              for t in range(NTC)]
        for t in range(NTC):
            ones_dst = vt[t][:].rearrange("p (h c) -> p h c", c=EV)[:, :, E]
            nc.vector.memset(ones_dst, 1.0)
        pvg = [proj.tile([P, 512], bf16, tag=f"pvg{t}", name=f"pvg{t}")
               for t in range(NTC)]

        aux = ctx.enter_context(tc.tile_pool(name="aux", bufs=2, space="PSUM"))
        sps = ctx.enter_context(tc.tile_pool(name="sps", bufs=2, space="PSUM"))
        pvqp = ctx.enter_context(tc.tile_pool(name="pvq", bufs=1, space="PSUM"))
        upool = ctx.enter_context(tc.tile_pool(name="upool", bufs=4))
        npool = ctx.enter_context(tc.tile_pool(name="npool", bufs=8))
        dtpa = ctx.enter_context(tc.tile_pool(name="dtpa", bufs=1))

        # ---- V / QK projection machinery ----------------------------------
        # LIFO pool discipline: vstack (closed at pair 1) sits on top of
        # qkstack (closed at the end), which sits on the long-lived pools.
        qkstack = ExitStack()
        wqkp = qkstack.enter_context(tc.tile_pool(name="wqk", bufs=1))
        xqkp = qkstack.enter_context(tc.tile_pool(name="xsqk", bufs=2))
        vstack = ExitStack()
        wvp = vstack.enter_context(tc.tile_pool(name="wv", bufs=1))
        xvp = vstack.enter_context(tc.tile_pool(name="xsv", bufs=2))

        wv = wvp.tile([P, 8 * 512], bf16, tag="wv")
        wq = wqkp.tile([P, 8 * 512], bf16, tag="wq")
        wk = wqkp.tile([P, 8 * 512], bf16, tag="wk")
        for hh in range(2):
            nc.sync.dma_start(
                wv[:, 2048 * hh:2048 * (hh + 1)].rearrange(
                    "p (d q) -> p d q", q=512),
                dram_chunks(wvT, ND, slice(0, 512))[:, 4 * hh:4 * (hh + 1), :])

        xvb = {}

        def load_v_block(tb):
            cols = slice(TQB * tb, TQB * (tb + 1))
            xb = xvp.tile([P, 8 * 512], bf16, tag="xsv")
            for hh in range(2):
                nc.sync.dma_start(
                    xb[:, 2048 * hh:2048 * (hh + 1)].rearrange(
                        "p (d q) -> p d q", q=512),
                    dram_chunks(xvT, ND, cols)[:, 4 * hh:4 * (hh + 1), :])
            xvb[tb] = xb

        def v_group(t):
            """Project V for one 128-row t-chunk into vt[t]."""
            tb, tc_ = t // 4, t % 4
            xb = xvb[tb]
            ps = aux.tile([P, 512], f32, tag="aux", name="psv")
            for d in range(ND):
                nc.tensor.matmul(
                    ps[:], xb[:, 512 * d + P * tc_:512 * d + P * (tc_ + 1)],
                    wv[:, 512 * d:512 * (d + 1)],
                    start=(d == 0), stop=(d == ND - 1))
            dst = vt[t][:].rearrange("p (h c) -> p h c", c=EV)[:, :, 0:E]
            src = ps[:].rearrange("p (h c) -> p h c", c=E)
            nc.vector.tensor_tensor(
                dst, src, bvtt[:].rearrange("p (h c) -> p h c", c=E), op=add_op)
            if tc_ == 3 and tb + 2 < NTQ:
                load_v_block(tb + 2)   # after tb's last consumer of the slot

        xqkb = {}

        def load_qk_block(tb):
            cols = slice(TQB * tb, TQB * (tb + 1))
            xb = xqkp.tile([P, 16 * 512], bf16, tag="xsqk", name="xqkb")
            for src, koff in ((xqT, 0), (xkT, 8 * 512)):
                for hh in range(2):
                    nc.sync.dma_start(
                        xb[:, koff + 2048 * hh:koff + 2048 * (hh + 1)].rearrange(
                            "p (d q) -> p d q", q=512),
                        dram_chunks(src, ND, cols)[:, 4 * hh:4 * (hh + 1), :])
            xqkb[tb] = xb

        def qk_group(tb, p):
            """Project Q and K of pair p for one 512-col t-block (with bias)."""
            xb = xqkb[tb]
            for wt, koff, dest, bcol in ((wq, 0, qt, p), (wk, 8 * 512, kt, 4 + p)):
                ps = aux.tile([P, 512], f32, tag="aux", name="psqk")
                for d in range(ND):
                    nc.tensor.matmul(
                        ps[:], wt[:, 512 * d + P * p:512 * d + P * (p + 1)],
                        xb[:, koff + 512 * d:koff + 512 * (d + 1)],
                        start=(d == 0), stop=(d == ND - 1))
                nc.scalar.activation(dest[p][:, TQB * tb:TQB * (tb + 1)], ps[:],
                                     Ident, bias=bqkt[:, bcol:bcol + 1])
            if p == 3 and tb + 2 < NTQ:
                load_qk_block(tb + 2)   # after tb's last consumer of the slot

        # ---- inline prologue: V(t 0-7) and QK(pair 0, tb 0) ---------------
        # DMA order puts the tensors the head needs first; small constants
        # slot in where their 0.6us issue cost is off the critical chain.
        load_v_block(0)
        bvtt = consts.tile([P, 512], bf16, tag="bvtt")
        nc.sync.dma_start(bvtt[:], bvt[:])   # needed by the first V copy
        load_qk_block(0)
        for wt, src in ((wq, wqT), (wk, wkT)):
            for hh in range(2):
                nc.sync.dma_start(
                    wt[:, 2048 * hh:2048 * (hh + 1)].rearrange(
                        "p (d q) -> p d q", q=512),
                    dram_chunks(src, ND, slice(0, 512))[:, 4 * hh:4 * (hh + 1), :])
        bqkt = consts.tile([P, 8], f32, tag="bqkt")
        nc.sync.dma_start(bqkt[:], bqk[:])   # needed by the first QK bias-add
        load_v_block(1)
        load_qk_block(1)
        mskt = consts.tile([P, 256], bf16, tag="mskt")
        nc.sync.dma_start(mskt[:], msk[:])   # needed at the first diag chunk
        for t in range(4):
            v_group(t)
        qk_group(0, 0)

        # deferred work, drip-fed into the attention chunk stream
        vthunks = [(lambda t=t: v_group(t)) for t in range(4, NTC)]
        qkthunks = [(lambda tb=tb, p=p: qk_group(tb, p))
                    for tb in range(NTQ) for p in range(4) if (tb, p) != (0, 0)]
        workq = []   # FIFO of D@PV thunks

        def drip(n):
            for _ in range(n):
                if vthunks:
                    vthunks.pop(0)()
                elif qkthunks:
                    qkthunks.pop(0)()
                elif workq:
                    workq.pop(0)()
                else:
                    break

        # ---- attention + drip-fed projections / D@PV ----------------------
        a2s = []     # filled when the a2s pool opens (pair 1)
        dTt = []     # D^T resident tiles  (pool opens at pair 0 start)

        def dT_slice(t, qb):
            return dTt[t // 4][:, T * (t % 4) + TQB * qb:T * (t % 4) + TQB * (qb + 1)]

        def dpv_pieces(p, qb):
            """D@PV for (pair, q-block) as 5 drip-sized thunks."""
            st = {}

            def mm(t0):
                def run():
                    if t0 == 0:
                        st["a2"] = aux.tile([P, 512], f32, tag="aux", name="a2")
                    for t in range(t0, t0 + 4):
                        nc.tensor.matmul(st["a2"][:], pvg[t][:, P * p:P * (p + 1)],
                                         dT_slice(t, qb),
                                         start=(t == 0), stop=(t == NTC - 1))
                return run

            def cp():
                nc.vector.tensor_copy(a2s[p][qb][:], st["a2"][:])
            return [mm(0), mm(4), mm(8), mm(12), cp]

        def dpv_thunk(p, qb):
            def run():
                for piece in dpv_pieces(p, qb):
                    piece()
            return run

        late = ExitStack()
        prev_dpv = []

        if True:
            for p in range(4):
                # -- pair-boundary bookkeeping -----------------------------
                if p == 0:
                    # first half of D^T: loaded during pair 0 (DMA slack)
                    for t4 in range(2):
                        dte = dtpa.tile([P, 4 * T], bf16, tag=f"dT{t4}",
                                        name=f"dT{t4}")
                        nc.sync.dma_start(
                            dte[:].rearrange("p (c q) -> p c q", q=T),
                            dTd[P * 4 * t4:P * 4 * (t4 + 1), :].rearrange(
                                "(c p) q -> p c q", p=P))
                        dTt.append(dte)
                if p == 1:
                    while vthunks or qkthunks:   # all proj done; free pools
                        drip(1)
                    vstack.close()
                    qkstack.close()
                    dtpb = late.enter_context(tc.tile_pool(name="dtpb", bufs=1))
                    for t4 in range(2, 4):
                        dte = dtpb.tile([P, 4 * T], bf16, tag=f"dT{t4}",
                                        name=f"dT{t4}")
                        nc.sync.dma_start(
                            dte[:].rearrange("p (c q) -> p c q", q=T),
                            dTd[P * 4 * t4:P * 4 * (t4 + 1), :].rearrange(
                                "(c p) q -> p c q", p=P))
                        dTt.append(dte)
                    wop = late.enter_context(tc.tile_pool(name="wop", bufs=1))
                    woh.append(wop.tile([P, 4 * D], bf16, tag="wo", name="wo"))
                    nc.sync.dma_start(woh[0][:].rearrange("p (c q) -> p c q", q=D),
                                      dram_chunks(woT, 4, slice(0, D)))
                    a2sp = late.enter_context(tc.tile_pool(name="a2s", bufs=1))
                    for pp in range(4):
                        a2s.append([a2sp.tile([P, 512], bf16, tag=f"a2s{pp}_{qb}",
                                              name=f"a2s{pp}_{qb}")
                                    for qb in range(NTQ)])
                workq.extend(prev_dpv)
                prev_dpv = []

                # -- attention for pair p ----------------------------------
                for i in range(NTQ):
                    if p == 0:
                        while 15 - len(qkthunks) < 4 * i:  # (tb=i, p0) needed
                            qkthunks.pop(0)()
                    nch = 4 * (i + 1)
                    pvqt = [pvqp.tile([P, 512], f32, tag="pvqA", name="pvqA"),
                            pvqp.tile([P, 512], f32, tag="pvqB", name="pvqB")]
                    for c in range(nch):
                        if p == 0:
                            while len(vthunks) > NTC - 4 - max(0, c - 3):
                                vthunks.pop(0)()   # vt[c] must exist for PV
                        jj = c - 4 * i           # >= 0 on the diagonal block
                        off = max(0, P * jj)     # skip q < off (above diag)
                        sp = sps.tile([P, 1024], f32, tag="sp")
                        for h in range(2):
                            nc.tensor.matmul(
                                sp[:, 512 * h + off:512 * (h + 1)],
                                kt[p][E * h:E * (h + 1), P * c:P * (c + 1)],
                                qt[p][E * h:E * (h + 1),
                                      TQB * i + off:TQB * (i + 1)],
                                start=True, stop=True)
                        u = upool.tile([P, 1024], bf16, tag="u")
                        u3 = u[:].rearrange("p (h q) -> p h q", q=TQB)
                        sp3 = sp[:].rearrange("p (h q) -> p h q", q=TQB)
                        nc.scalar.activation(u3[:, :, off:TQB], sp3[:, :, off:TQB],
                                             Exp, scale=COEF)
                        if jj >= 0:
                            # mask the 128x128 diagonal block of each head
                            nc.vector.tensor_tensor(
                                u3[:, :, off:off + P], u3[:, :, off:off + P],
                                mskt[:].rearrange("p (h q) -> p h q", q=P),
                                op=mult)
                        if p == 0 or i >= 2:
                            drip(1)   # cover the exp latency with queued work
                        if c == 0:
                            for pv in pvqt:
                                # one start=True matmul zeroes the bank
                                # (has_written is bank-granular); the (h, qs)
                                # sub-groups then accumulate on top of it.
                                nc.tensor.matmul(pv[:, 0:260], zrow[0:1, 0:P],
                                                 zrow[0:1, 0:260],
                                                 start=True, stop=False,
                                                 skip_group_check=True)
                        for h in range(2):
                            vcol = EV * (2 * p + h)
                            for qs in range(max(0, jj), 4):
                                col0 = 130 * (qs % 2) + 65 * h
                                nc.tensor.matmul(
                                    pvqt[qs // 2][:, col0:col0 + EV],
                                    u[:, 512 * h + P * qs:512 * h + P * (qs + 1)],
                                    vt[c][:, vcol:vcol + EV],
                                    start=False, stop=(c == 4 * i + qs),
                                    skip_group_check=True)
                        if jj in (1, 3):
                            # this pvq bank got its last write: normalize both
                            # of its q-subs (bank-level PE-W/DVE-R separation)
                            half = jj // 2
                            pv = pvqt[half]
                            rcp = npool.tile([P, 4], f32, tag="rcp")
                            sums = pv[:, 0:260].rearrange(
                                "p (s c) -> p s c", c=65)[:, :, E]
                            nc.vector.reciprocal(rcp[:], sums)
                            for ql in range(2):
                                qs = 2 * half + ql
                                for h in range(2):
                                    col0 = 130 * ql + 65 * h
                                    nc.vector.tensor_scalar(
                                        pvg[4 * i + qs][:,
                                                        E * (2 * p + h):E * (2 * p + h + 1)],
                                        pv[:, col0:col0 + E],
                                        rcp[:, 2 * ql + h:2 * ql + h + 1],
                                        None, op0=mult)
                    drip(1)
                # -- D@PV of this pair, drip-fed into the next pair --------
                if p < 3:
                    prev_dpv = [t for qb in range(NTQ) for t in dpv_pieces(p, qb)]
                else:
                    while vthunks or qkthunks or workq:
                        drip(1)
                    # fused D@PV(pair 3) + out-projection tail
                    with tc.tile_pool(name="obuf", bufs=3) as obp:
                        for qb in range(NTQ):
                            dpv_thunk(3, qb)()
                            for qs in range(4):
                                ob = obp.tile([P, 1024], f32, tag="ob")
                                for nh in range(2):
                                    op_ = aux.tile([P, 512], f32, tag="aux",
                                                   name="op")
                                    for cc in range(4):
                                        nc.tensor.matmul(
                                            op_[:],
                                            a2s[cc][qb][:, P * qs:P * (qs + 1)],
                                            woh[0][:, D * cc + 512 * nh:
                                                   D * cc + 512 * (nh + 1)],
                                            start=(cc == 0), stop=(cc == 3))
                                    if nh == 0:
                                        nc.scalar.copy(
                                            ob[:, 512 * nh:512 * (nh + 1)], op_[:])
                                    else:
                                        nc.vector.tensor_copy(
                                            ob[:, 512 * nh:512 * (nh + 1)], op_[:])
                                    nc.sync.dma_start(
                                        y[TQB * qb + P * qs:
                                          TQB * qb + P * (qs + 1),
                                          512 * nh:512 * (nh + 1)],
                                        ob[:, 512 * nh:512 * (nh + 1)])
        if dbg:
            with tc.tile_pool(name="dbgp", bufs=2) as dbp:
                def dump(dst_rows, tileap, dram, w):
                    tmp = dbp.tile([P, w], f32, tag="dbg", name="dbg")
                    nc.vector.tensor_copy(tmp[:, 0:tileap.shape[-1]], tileap)
                    nc.sync.dma_start(
                        dram[dst_rows:dst_rows + P, 0:tileap.shape[-1]],
                        tmp[:, 0:tileap.shape[-1]])
                for p in range(4):
                    dump(128 * p, qt[p][:], dq, T)
                    dump(128 * p, kt[p][:], dk, T)
                for t in range(NTC):
                    dump(128 * t, vt[t][:], dv, HG * EV)
                    dump(128 * t, pvg[t][:], dpvg, 512)
                for pp in range(4):
                    for qb in range(NTQ):
                        dump(128 * (4 * pp + qb) if False else
                             512 * pp + 128 * qb, a2s[pp][qb][:], da2, 512)
        late.close()
        qkstack.close()

    nc.compile()
    return nc


def _prep_inputs(query_1, key_1, value_1, Wq, bq, Wk, bk, Wv, bv, Wo, bo, Dmat):
    """Host-side sharding: per-core input dicts (bf16 operands)."""
    import ml_dtypes
    f = np.float32
    bf = ml_dtypes.bfloat16
    ones_row = np.ones((1, T), f)

    def xT(x, b):
        return np.ascontiguousarray(
            np.vstack([np.asarray(x[b], f).T, ones_row]).astype(bf))

    # per head-group weights
    wqTs, wkTs, wvTs, woTs, bqks, bvts = [], [], [], [], [], []
    for g in range(2):
        h0 = HG * g
        wq = np.zeros((D, 512), f)
        wk = np.zeros((D, 512), f)
        bqkm = np.zeros((P, 8), f)
        for p in range(4):
            for h in range(2):
                hh = h0 + 2 * p + h
                c0 = 128 * p + 64 * h
                wq[:, c0:c0 + 64] = np.asarray(Wq[hh], f).T
                wk[:, c0:c0 + 64] = np.asarray(Wk[hh], f).T
                bqkm[64 * h:64 * (h + 1), p] = np.asarray(bq[hh], f)
                bqkm[64 * h:64 * (h + 1), 4 + p] = np.asarray(bk[hh], f)
        wv = np.zeros((D + 1, 512), f)
        for jh in range(HG):
            wv[:D, 64 * jh:64 * (jh + 1)] = np.asarray(Wv[h0 + jh], f).T
            wv[D, 64 * jh:64 * (jh + 1)] = np.asarray(bv[h0 + jh], f)
        wo = np.ascontiguousarray(np.asarray(Wo, f)[:, 64 * h0:64 * (h0 + HG)].T)
        bvm = np.tile(np.asarray(bv[h0:h0 + HG], f).reshape(1, 512), (P, 1))
        wqTs.append(wq.astype(bf))
        bvts.append(bvm.astype(bf))
        wkTs.append(wk.astype(bf))
        wvTs.append(wv.astype(bf))
        woTs.append(wo.astype(bf))
        bqks.append(bqkm)

    dT = np.ascontiguousarray(np.asarray(Dmat, f).T).astype(bf)
    tri = (np.arange(128)[:, None] <= np.arange(128)[None, :]).astype(f)
    msk = np.ascontiguousarray(np.tile(tri, (1, 2))).astype(bf)  # [128, 256]

    xqTs = [xT(query_1, b) for b in range(B)]
    xkTs = [xT(key_1, b) for b in range(B)]
    xvTs = [xT(value_1, b) for b in range(B)]

    in_maps = []
    for c in range(8):
        b, g = c // 2, c % 2
        in_maps.append({
            "xqT": xqTs[b], "xkT": xkTs[b], "xvT": xvTs[b],
            "wqT": wqTs[g], "wkT": wkTs[g], "wvT": wvTs[g], "woT": woTs[g],
            "dTd": dT, "msk": msk, "bqk": bqks[g], "bvt": bvts[g],
        })
    return in_maps


def kernel(query_1, key_1, value_1, Wq, bq, Wk, bk, Wv, bv, Wo, bo, D):
    import os
    os.environ["BASS_NEVER_TRACE"] = "1"  # NTFF capture hangs over the axon relay
    global _CACHED_NC
    if _CACHED_NC is None:
        _CACHED_NC = _build_nc()
    nc = _CACHED_NC
    in_maps = _prep_inputs(query_1, key_1, value_1, Wq, bq, Wk, bk, Wv, bv, Wo, bo, D)
    res = run_bass_kernel_spmd(nc, in_maps, core_ids=list(range(8)))
    bo_f = np.asarray(bo, np.float32)
    out = np.empty((B, T, 1024), np.float32)
    for b in range(B):
        out[b] = res.results[2 * b]["y"] + res.results[2 * b + 1]["y"] + bo_f
    return out
